# revision 13
# baseline (speedup 1.0000x reference)
"""Causal single-head attention (B=8, S=E=1024) for 8 Trainium2 cores.

Strategy: data-parallel over batch — core b handles batch element b.
All matmul operands are bf16 (1.0 cyc/row on the PE like fp32r, but half
the DMA/SBUF traffic, no ap<256 fp32r penalty, cheap transposes, 2x DVE).
PSUM accumulation stays fp32; measured end-to-end rel err ~6e-3 vs fp32.

Phase order (PE never waits on softmax):
  1. kT[d,s] = WkT.T @ xT (+bk)     boot chunk eo-outer over 8 PSUM banks
  2. qT[d,s] = (Wq/32)T.T @ xT (+bq/32)  (1/32 folded into host weights)
  3. ALL scores_i = qT_i.T @ kT -> diag mask -> p_i = exp(scores) in bf16
     with fused row-sum; the exp for tile i runs on Act while the PE is
     already on tile i+1 (no max subtraction: |scores| <= ~8, fp32-safe)
  4. V[s,d] = xT.T @ WvT (+bv)     (exp tail drains under the V matmuls)
  5. per i: PE-transpose p_i blocks -> AV accumulate -> *1/l -> DMA out

DMA plan: three HWDGE rings (~88 B/ns each, ~0.7us issue cost per
dma_start). Scalar ring only carries early-x blocks (drains before the
first PSUM evictions); W chunks split across rings sized so arrival
stays ahead of the PE's 13.6us/chunk consumption. Output tiles
alternate sync/gpsimd; the final tile is split 4 ways to shorten the
end-of-kernel drain.
"""

import os
import sys
from contextlib import ExitStack

for _p in ("/opt/trn_rl_repo", "/root/.axon_site/_ro/trn_rl_repo"):
    if os.path.isdir(_p) and _p not in sys.path:
        sys.path.insert(0, _p)

import numpy as np
import ml_dtypes

import concourse.bass as bass
import concourse.mybir as mybir
import concourse.tile as tile
from concourse import bacc
from concourse.bass_utils import run_bass_kernel_spmd
from concourse.masks import make_causal_mask, make_identity

P = 128
S = 1024
E = 1024
D = 1024
B = 8
SO = S // P
EO = E // P
DO = D // P
CH = 512
NCH = D // CH
SCALE = 1.0 / np.sqrt(float(E))  # 1/32
MASK_VAL = -1e9

F32 = mybir.dt.float32
BF16 = mybir.dt.bfloat16


def build_program():
    nc = bacc.Bacc(
        "TRN2", target_bir_lowering=False, debug=False, enable_asserts=True
    )

    xT = nc.dram_tensor("xT", [E, S], BF16, kind="ExternalInput").ap()
    wqT = nc.dram_tensor("wqT", [E, D], BF16, kind="ExternalInput").ap()  # *1/32
    wkT = nc.dram_tensor("wkT", [E, D], BF16, kind="ExternalInput").ap()
    wvT = nc.dram_tensor("wvT", [E, D], BF16, kind="ExternalInput").ap()
    bqs = nc.dram_tensor("bqs", [D], F32, kind="ExternalInput").ap()  # bq/32
    bk = nc.dram_tensor("bk", [D], F32, kind="ExternalInput").ap()
    bv = nc.dram_tensor("bv", [D], F32, kind="ExternalInput").ap()
    out = nc.dram_tensor("out", [S, D], BF16, kind="ExternalOutput").ap()

    with tile.TileContext(nc) as tc, ExitStack() as ctx:
        consts = ctx.enter_context(tc.tile_pool(name="consts", bufs=1))
        bigs = ctx.enter_context(tc.tile_pool(name="bigs", bufs=1))
        wpool = ctx.enter_context(tc.tile_pool(name="wpool", bufs=2))
        small = ctx.enter_context(tc.tile_pool(name="small", bufs=16))

        # resident tensors (all bf16)
        x_sb = bigs.tile([P, EO, S], BF16)
        kT_sb = bigs.tile([P, DO, S], BF16)
        qT_sb = bigs.tile([P, DO, S], BF16)
        v_sb = bigs.tile([P, SO, D], BF16)
        p_all = bigs.tile([P, SO, S], BF16)  # exp(scores) for every q-tile

        wq_r = wqT.rearrange("(eo p) o -> p eo o", p=P)
        wk_r = wkT.rearrange("(eo p) o -> p eo o", p=P)
        wv_r = wvT.rearrange("(eo p) o -> p eo o", p=P)

        # ---- startup: x / wk-chunk0 spread across all three DMA rings ----
        # Each ring moves ~88 B/ns and a dma_start costs ~0.7us of issue time
        # on the owning engine's in-order queue, so: finest grains first, the
        # Scalar ring only carries work that drains before evictions begin,
        # and blocks are ordered to track the boot's eo-consumption.
        wk0_pool = ctx.enter_context(tc.tile_pool(name="wk0_pool", bufs=1))
        wk0 = wk0_pool.tile([P, EO, CH], BF16, name="wk0")
        x_r = xT.rearrange("(eo p) s -> p eo s", p=P)
        nc.sync.dma_start(wk0[:, 0, :], wk_r[:, 0, 0:CH])
        nc.scalar.dma_start(x_sb[:, 0, 0:CH], x_r[:, 0, 0:CH])
        nc.gpsimd.dma_start(x_sb[:, 0, CH:S], x_r[:, 0, CH:S])
        nc.sync.dma_start(x_sb[:, 1, :], x_r[:, 1, :])
        nc.scalar.dma_start(x_sb[:, 2, :], x_r[:, 2, :])
        nc.gpsimd.dma_start(wk0[:, 1, :], wk_r[:, 1, 0:CH])
        nc.gpsimd.dma_start(wk0[:, 2:4, :], wk_r[:, 2:4, 0:CH])
        nc.sync.dma_start(x_sb[:, 3, :], x_r[:, 3, :])
        nc.scalar.dma_start(x_sb[:, 4, :], x_r[:, 4, :])
        nc.gpsimd.dma_start(wk0[:, 4:6, :], wk_r[:, 4:6, 0:CH])
        nc.sync.dma_start(x_sb[:, 6, :], x_r[:, 6, :])
        nc.scalar.dma_start(x_sb[:, 5, :], x_r[:, 5, :])
        nc.gpsimd.dma_start(wk0[:, 6:8, :], wk_r[:, 6:8, 0:CH])
        nc.scalar.dma_start(x_sb[:, 7, :], x_r[:, 7, :])

        # small consts on the pool ring (needed by ~first eviction)
        bq_t = consts.tile([P, DO], F32)
        nc.gpsimd.dma_start(bq_t, bqs.rearrange("(o p) -> p o", p=P))
        bk_t = consts.tile([P, DO], F32)
        nc.gpsimd.dma_start(bk_t, bk.rearrange("(o p) -> p o", p=P))
        identity = consts.tile([P, P], BF16)
        make_identity(nc, identity)
        cmask = consts.tile([P, P], F32)
        make_causal_mask(nc, cmask, mask_val=MASK_VAL)
        # bv broadcast across partitions (needed only for V evictions, late)
        bv_b = consts.tile([P, D], F32)
        nc.gpsimd.dma_start(bv_b, bv[None, :].broadcast_to([P, D]))

        # PE warmup while the first DMAs are in flight: the tensor engine
        # ramps its p-state only after sustained work, so burn ~1.5us on
        # dummy transposes of the (already generated) identity tile to hit
        # full clock before the first real matmul.
        with tc.tile_pool(name="warm_ps", bufs=1, space="PSUM") as warm_ps:
            wps = warm_ps.tile([P, P], BF16, tag="warm", name="warm")
            for _ in range(12):
                nc.tensor.transpose(wps, identity, identity)

        # ---- kT chunk 0: eo-outer over 8 simultaneously-open psum banks ----
        with tc.tile_pool(name="boot_ps", bufs=8, space="PSUM") as boot_ps:
            groups = [(dj, ch) for dj in range(CH // P) for ch in range(S // CH)]
            boot_tiles = [
                boot_ps.tile([P, CH], F32, tag="boot", name=f"bps_{g}")
                for g in range(len(groups))
            ]
            for eo in range(EO):
                for g, (dj, ch) in enumerate(groups):
                    nc.tensor.matmul(
                        boot_tiles[g],
                        lhsT=wk0[:, eo, dj * P : (dj + 1) * P],
                        rhs=x_sb[:, eo, ch * CH : (ch + 1) * CH],
                        start=(eo == 0),
                        stop=(eo == EO - 1),
                    )
            for g, (dj, ch) in enumerate(groups):
                if g % 2 == 0:
                    nc.scalar.activation(
                        kT_sb[:, dj, ch * CH : (ch + 1) * CH],
                        boot_tiles[g],
                        mybir.ActivationFunctionType.Identity,
                        bias=bk_t[:, dj : dj + 1],
                        scale=1.0,
                    )
                else:
                    nc.vector.tensor_scalar(
                        kT_sb[:, dj, ch * CH : (ch + 1) * CH],
                        boot_tiles[g],
                        bk_t[:, dj : dj + 1],
                        None,
                        mybir.AluOpType.add,
                    )

        ppool = ctx.enter_context(tc.tile_pool(name="ppool", bufs=2))
        acc_ps = ctx.enter_context(tc.tile_pool(name="acc_ps", bufs=2, space="PSUM"))
        sc_ps = ctx.enter_context(tc.tile_pool(name="sc_ps", bufs=2, space="PSUM"))
        tr_ps = ctx.enter_context(tc.tile_pool(name="tr_ps", bufs=2, space="PSUM"))

        def load_w_chunk(w_r, c, nm, rings=(nc.sync, nc.gpsimd)):
            # split across two rings; one ring alone (~11.6us/MB) barely keeps
            # up with the PE's 13.6us per-chunk consumption
            wt = wpool.tile([P, EO, CH], BF16, tag="wchunk", name=nm)
            rings[0].dma_start(wt[:, 0:4, :], w_r[:, 0:4, c * CH : (c + 1) * CH])
            rings[1].dma_start(wt[:, 4:8, :], w_r[:, 4:8, c * CH : (c + 1) * CH])
            return wt

        def project_chunk(wt, c, dst, bias_t):
            # dst[d_part, do, s] (+bias per-partition), for d in chunk c
            for dj in range(CH // P):
                do = c * (CH // P) + dj
                for ch in range(S // CH):
                    ps = acc_ps.tile([P, CH], F32, tag="acc", name="ps")
                    for eo in range(EO):
                        nc.tensor.matmul(
                            ps,
                            lhsT=wt[:, eo, dj * P : (dj + 1) * P],
                            rhs=x_sb[:, eo, ch * CH : (ch + 1) * CH],
                            start=(eo == 0),
                            stop=(eo == EO - 1),
                        )
                    nc.scalar.activation(
                        dst[:, do, ch * CH : (ch + 1) * CH],
                        ps,
                        mybir.ActivationFunctionType.Identity,
                        bias=bias_t[:, do : do + 1],
                        scale=1.0,
                    )

        # rest of kT, then qT (scores need them first).  wk1 rides the
        # scalar+sync rings (both drain before evictions start); later chunks
        # use sync+gpsimd to keep the Scalar queue free for activations.
        wt = load_w_chunk(wk_r, 1, "wk1", rings=(nc.scalar, nc.sync))
        project_chunk(wt, 1, kT_sb, bk_t)
        for c in range(NCH):
            wt = load_w_chunk(wq_r, c, f"wq{c}")
            project_chunk(wt, c, qT_sb, bq_t)

        # ---- all scores + exp (Act drains one tile behind the PE) ----
        rinvs = []
        for i in range(SO):
            nk = i + 1
            kw = nk * P
            ps_s = sc_ps.tile([P, S], F32, tag="scores", name="ps_s")
            nfull = kw // CH
            rem = kw - nfull * CH
            for ch in range(nfull + (1 if rem else 0)):
                w = CH if ch < nfull else rem
                for do in range(DO):
                    nc.tensor.matmul(
                        ps_s[:, ch * CH : ch * CH + w],
                        lhsT=qT_sb[:, do, i * P : (i + 1) * P],
                        rhs=kT_sb[:, do, ch * CH : ch * CH + w],
                        start=(do == 0),
                        stop=(do == DO - 1),
                    )
            # additive causal mask on the diagonal block
            nc.vector.tensor_tensor(
                ps_s[:, i * P : (i + 1) * P],
                ps_s[:, i * P : (i + 1) * P],
                cmask,
                mybir.AluOpType.add,
            )
            # p = exp(scores); no max subtraction; fused row-sum
            lsum = small.tile([P, 1], F32, tag="lsum", name=f"lsum{i}")
            nc.scalar.activation(
                p_all[:, i, 0:kw],
                ps_s[:, :kw],
                mybir.ActivationFunctionType.Exp,
                bias=0.0,
                scale=1.0,
                accum_out=lsum,
            )
            rinv = small.tile([P, 1], F32, tag="rinv", name=f"rinv{i}")
            nc.vector.reciprocal(rinv, lsum)
            rinvs.append(rinv)

        # ---- V[s_part, so, d] = x.T @ WvT (+bv along free dim) ----
        for c in range(NCH):
            wt = load_w_chunk(wv_r, c, f"wv{c}")
            for so in range(SO):
                ps = acc_ps.tile([P, CH], F32, tag="acc", name="ps")
                for eo in range(EO):
                    nc.tensor.matmul(
                        ps,
                        lhsT=x_sb[:, eo, so * P : (so + 1) * P],
                        rhs=wt[:, eo, :],
                        start=(eo == 0),
                        stop=(eo == EO - 1),
                    )
                nc.vector.tensor_tensor(
                    v_sb[:, so, c * CH : (c + 1) * CH],
                    ps,
                    bv_b[:, c * CH : (c + 1) * CH],
                    mybir.AluOpType.add,
                )

        # ---- transpose + AV per q-tile (all deps already resolved) ----
        # Descending i: the big early tiles hide the transpose->copy->AV
        # pipeline-fill latency, and the kernel ends on the smallest tile so
        # the final evict+DMA chain is short.
        for i in reversed(range(SO)):
            nk = i + 1
            rinv = rinvs[i]
            pT = ppool.tile([P, S], BF16, tag="pT", name="pT")
            for j in range(nk):
                ps_t = tr_ps.tile([P, P], BF16, tag="tr", name="ps_t")
                nc.tensor.transpose(ps_t, p_all[:, i, j * P : (j + 1) * P], identity)
                nc.vector.tensor_copy(pT[:, j * P : (j + 1) * P], ps_t)

            out_sb = ppool.tile([P, D], BF16, tag="out", name="out_sb")
            last = i == 0
            for c2 in range(NCH):
                ps_o = acc_ps.tile([P, CH], F32, tag="acc", name="ps_o")
                for j in range(nk):
                    nc.tensor.matmul(
                        ps_o,
                        lhsT=pT[:, j * P : (j + 1) * P],
                        rhs=v_sb[:, j, c2 * CH : (c2 + 1) * CH],
                        start=(j == 0),
                        stop=(j == nk - 1),
                    )
                base = c2 * CH
                if last and c2 == NCH - 1:
                    # final tile: split eviction across DVE+Act, one DMA
                    # issue per output ring, to shorten the tail drain
                    half = CH // 2
                    nc.vector.tensor_scalar_mul(
                        out_sb[:, base : base + half], ps_o[:, 0:half], rinv
                    )
                    nc.scalar.activation(
                        out_sb[:, base + half : base + CH],
                        ps_o[:, half:CH],
                        mybir.ActivationFunctionType.Identity,
                        bias=0.0,
                        scale=rinv,
                    )
                    nc.gpsimd.dma_start(
                        out[i * P : (i + 1) * P, base : base + half],
                        out_sb[:, base : base + half],
                    )
                    nc.sync.dma_start(
                        out[i * P : (i + 1) * P, base + half : base + CH],
                        out_sb[:, base + half : base + CH],
                    )
                else:
                    # alternate evictions DVE/Act and output rings
                    if c2 % 2 == 0:
                        nc.vector.tensor_scalar_mul(
                            out_sb[:, base : base + CH], ps_o, rinv
                        )
                    else:
                        nc.scalar.activation(
                            out_sb[:, base : base + CH],
                            ps_o,
                            mybir.ActivationFunctionType.Identity,
                            bias=0.0,
                            scale=rinv,
                        )
                    ring = nc.gpsimd if (i * NCH + c2) % 2 == 0 else nc.sync
                    ring.dma_start(
                        out[i * P : (i + 1) * P, base : base + CH],
                        out_sb[:, base : base + CH],
                    )

    nc.compile()
    return nc


_NC_CACHE = None


def get_program():
    global _NC_CACHE
    if _NC_CACHE is None:
        _NC_CACHE = build_program()
    return _NC_CACHE


def make_in_maps(x, Wq, bq, Wk, bk, Wv, bv):
    x = np.asarray(x, dtype=np.float32)
    wqT = np.ascontiguousarray(
        (np.asarray(Wq, dtype=np.float32).T * np.float32(SCALE)).astype(
            ml_dtypes.bfloat16
        )
    )
    wkT = np.ascontiguousarray(np.asarray(Wk, dtype=np.float32).T.astype(ml_dtypes.bfloat16))
    wvT = np.ascontiguousarray(np.asarray(Wv, dtype=np.float32).T.astype(ml_dtypes.bfloat16))
    bqs = np.asarray(bq, dtype=np.float32) * np.float32(SCALE)
    bk = np.asarray(bk, dtype=np.float32)
    bv = np.asarray(bv, dtype=np.float32)
    in_maps = []
    for b in range(B):
        in_maps.append(
            {
                "xT": np.ascontiguousarray(x[b].T.astype(ml_dtypes.bfloat16)),
                "wqT": wqT,
                "wkT": wkT,
                "wvT": wvT,
                "bqs": bqs,
                "bk": bk,
                "bv": bv,
            }
        )
    return in_maps


def run_on_hw(in_maps, trace=False, **kwargs):
    nc = get_program()
    return run_bass_kernel_spmd(
        nc, in_maps, core_ids=list(range(B)), trace=trace, **kwargs
    )


def kernel(x, Wq, bq, Wk, bk, Wv, bv):
    in_maps = make_in_maps(x, Wq, bq, Wk, bk, Wv, bv)
    res = run_on_hw(in_maps)
    return np.stack(
        [res.results[b]["out"].astype(np.float32) for b in range(B)], axis=0
    )


# revision 16
# speedup vs baseline: 1.0276x; 1.0276x over previous
"""Causal single-head attention (B=8, S=E=1024) for 8 Trainium2 cores.

Strategy: data-parallel over batch — core b handles batch element b.
All matmul operands are bf16 (1.0 cyc/row on the PE like fp32r, but half
the DMA/SBUF traffic, no ap<256 fp32r penalty, cheap transposes, 2x DVE).
PSUM accumulation stays fp32; measured end-to-end rel err ~6e-3 vs fp32.

Phase order (PE never waits on softmax):
  1. kT[d,s] = WkT.T @ xT (+bk)     boot chunk eo-outer over 8 PSUM banks
  2. qT[d,s] = (Wq/32)T.T @ xT (+bq/32)  (1/32 folded into host weights)
  3. ALL scores_i = qT_i.T @ kT -> diag mask -> p_i = exp(scores) in bf16
     with fused row-sum; the exp for tile i runs on Act while the PE is
     already on tile i+1 (no max subtraction: |scores| <= ~8, fp32-safe)
  4. V[s,d] = xT.T @ WvT (+bv)     (exp tail drains under the V matmuls)
  5. per i: PE-transpose p_i blocks -> AV accumulate -> *1/l -> DMA out

DMA plan: three HWDGE rings (~88 B/ns each, ~0.7us issue cost per
dma_start). Scalar ring only carries early-x blocks (drains before the
first PSUM evictions); W chunks split across rings sized so arrival
stays ahead of the PE's 13.6us/chunk consumption. Output tiles
alternate sync/gpsimd; the final tile is split 4 ways to shorten the
end-of-kernel drain.
"""

import os
import sys
from contextlib import ExitStack

for _p in ("/opt/trn_rl_repo", "/root/.axon_site/_ro/trn_rl_repo"):
    if os.path.isdir(_p) and _p not in sys.path:
        sys.path.insert(0, _p)

import numpy as np
import ml_dtypes

import concourse.bass as bass
import concourse.mybir as mybir
import concourse.tile as tile
from concourse import bacc
from concourse.bass_utils import run_bass_kernel_spmd
from concourse.masks import make_causal_mask, make_identity

P = 128
S = 1024
E = 1024
D = 1024
B = 8
SO = S // P
EO = E // P
DO = D // P
CH = 512
NCH = D // CH
SCALE = 1.0 / np.sqrt(float(E))  # 1/32
MASK_VAL = -1e9

F32 = mybir.dt.float32
BF16 = mybir.dt.bfloat16


def build_program():
    nc = bacc.Bacc(
        "TRN2", target_bir_lowering=False, debug=False, enable_asserts=True
    )

    xT = nc.dram_tensor("xT", [E, S], BF16, kind="ExternalInput").ap()
    wqT = nc.dram_tensor("wqT", [E, D], BF16, kind="ExternalInput").ap()  # *1/32
    wkT = nc.dram_tensor("wkT", [E, D], BF16, kind="ExternalInput").ap()
    wvT = nc.dram_tensor("wvT", [E, D], BF16, kind="ExternalInput").ap()
    bqs = nc.dram_tensor("bqs", [D], F32, kind="ExternalInput").ap()  # bq/32
    bk = nc.dram_tensor("bk", [D], F32, kind="ExternalInput").ap()
    bv = nc.dram_tensor("bv", [D], F32, kind="ExternalInput").ap()
    out = nc.dram_tensor("out", [S, D], BF16, kind="ExternalOutput").ap()

    with tile.TileContext(nc) as tc, ExitStack() as ctx:
        consts = ctx.enter_context(tc.tile_pool(name="consts", bufs=1))
        bigs = ctx.enter_context(tc.tile_pool(name="bigs", bufs=1))
        wpool = ctx.enter_context(tc.tile_pool(name="wpool", bufs=2))
        small = ctx.enter_context(tc.tile_pool(name="small", bufs=16))

        # resident tensors (all bf16)
        x_sb = bigs.tile([P, EO, S], BF16)
        kT_sb = bigs.tile([P, DO, S], BF16)
        qT_sb = bigs.tile([P, DO, S], BF16)
        v_sb = bigs.tile([P, SO, D], BF16)
        p_all = bigs.tile([P, SO, S], BF16)  # exp(scores) for every q-tile

        wq_r = wqT.rearrange("(eo p) o -> p eo o", p=P)
        wk_r = wkT.rearrange("(eo p) o -> p eo o", p=P)
        wv_r = wvT.rearrange("(eo p) o -> p eo o", p=P)

        # ---- startup: x / wk-chunk0 spread across all three DMA rings ----
        # Each ring moves ~88 B/ns and a dma_start costs ~0.7us of issue time
        # on the owning engine's in-order queue, so: finest grains first, the
        # Scalar ring only carries work that drains before evictions begin,
        # and blocks are ordered to track the boot's eo-consumption.
        wk0_pool = ctx.enter_context(tc.tile_pool(name="wk0_pool", bufs=1))
        wk0 = wk0_pool.tile([P, EO, CH], BF16, name="wk0")
        x_r = xT.rearrange("(eo p) s -> p eo s", p=P)
        nc.sync.dma_start(wk0[:, 0, :], wk_r[:, 0, 0:CH])
        nc.scalar.dma_start(x_sb[:, 0, 0:CH], x_r[:, 0, 0:CH])
        nc.gpsimd.dma_start(x_sb[:, 0, CH:S], x_r[:, 0, CH:S])
        nc.sync.dma_start(x_sb[:, 1, :], x_r[:, 1, :])
        nc.scalar.dma_start(x_sb[:, 2, :], x_r[:, 2, :])
        nc.gpsimd.dma_start(wk0[:, 1, :], wk_r[:, 1, 0:CH])
        nc.gpsimd.dma_start(wk0[:, 2:4, :], wk_r[:, 2:4, 0:CH])
        nc.sync.dma_start(x_sb[:, 3, :], x_r[:, 3, :])
        nc.scalar.dma_start(x_sb[:, 4, :], x_r[:, 4, :])
        nc.gpsimd.dma_start(wk0[:, 4:6, :], wk_r[:, 4:6, 0:CH])
        nc.sync.dma_start(x_sb[:, 6, :], x_r[:, 6, :])
        nc.scalar.dma_start(x_sb[:, 5, :], x_r[:, 5, :])
        nc.gpsimd.dma_start(wk0[:, 6:8, :], wk_r[:, 6:8, 0:CH])
        nc.scalar.dma_start(x_sb[:, 7, :], x_r[:, 7, :])

        # small consts on the pool ring (needed by ~first eviction)
        bq_t = consts.tile([P, DO], F32)
        nc.gpsimd.dma_start(bq_t, bqs.rearrange("(o p) -> p o", p=P))
        bk_t = consts.tile([P, DO], F32)
        nc.gpsimd.dma_start(bk_t, bk.rearrange("(o p) -> p o", p=P))
        identity = consts.tile([P, P], BF16)
        make_identity(nc, identity)
        cmask = consts.tile([P, P], F32)
        make_causal_mask(nc, cmask, mask_val=MASK_VAL)
        # bv broadcast across partitions (needed only for V evictions, late)
        bv_b = consts.tile([P, D], F32)
        nc.gpsimd.dma_start(bv_b, bv[None, :].broadcast_to([P, D]))

        # ---- kT chunk 0: eo-outer over 8 simultaneously-open psum banks ----
        with tc.tile_pool(name="boot_ps", bufs=8, space="PSUM") as boot_ps:
            groups = [(dj, ch) for dj in range(CH // P) for ch in range(S // CH)]
            boot_tiles = [
                boot_ps.tile([P, CH], F32, tag="boot", name=f"bps_{g}")
                for g in range(len(groups))
            ]
            for eo in range(EO):
                for g, (dj, ch) in enumerate(groups):
                    nc.tensor.matmul(
                        boot_tiles[g],
                        lhsT=wk0[:, eo, dj * P : (dj + 1) * P],
                        rhs=x_sb[:, eo, ch * CH : (ch + 1) * CH],
                        start=(eo == 0),
                        stop=(eo == EO - 1),
                    )
            for g, (dj, ch) in enumerate(groups):
                if g % 2 == 0:
                    nc.scalar.activation(
                        kT_sb[:, dj, ch * CH : (ch + 1) * CH],
                        boot_tiles[g],
                        mybir.ActivationFunctionType.Identity,
                        bias=bk_t[:, dj : dj + 1],
                        scale=1.0,
                    )
                else:
                    nc.vector.tensor_scalar(
                        kT_sb[:, dj, ch * CH : (ch + 1) * CH],
                        boot_tiles[g],
                        bk_t[:, dj : dj + 1],
                        None,
                        mybir.AluOpType.add,
                    )

        ppool = ctx.enter_context(tc.tile_pool(name="ppool", bufs=2))
        acc_ps = ctx.enter_context(tc.tile_pool(name="acc_ps", bufs=2, space="PSUM"))
        sc_ps = ctx.enter_context(tc.tile_pool(name="sc_ps", bufs=2, space="PSUM"))
        tr_ps = ctx.enter_context(tc.tile_pool(name="tr_ps", bufs=2, space="PSUM"))

        def load_w_chunk(w_r, c, nm, rings=(nc.sync, nc.gpsimd)):
            # split across two rings; one ring alone (~11.6us/MB) barely keeps
            # up with the PE's 13.6us per-chunk consumption
            wt = wpool.tile([P, EO, CH], BF16, tag="wchunk", name=nm)
            rings[0].dma_start(wt[:, 0:4, :], w_r[:, 0:4, c * CH : (c + 1) * CH])
            rings[1].dma_start(wt[:, 4:8, :], w_r[:, 4:8, c * CH : (c + 1) * CH])
            return wt

        def project_chunk(wt, c, dst, bias_t):
            # dst[d_part, do, s] (+bias per-partition), for d in chunk c
            for dj in range(CH // P):
                do = c * (CH // P) + dj
                for ch in range(S // CH):
                    ps = acc_ps.tile([P, CH], F32, tag="acc", name="ps")
                    for eo in range(EO):
                        nc.tensor.matmul(
                            ps,
                            lhsT=wt[:, eo, dj * P : (dj + 1) * P],
                            rhs=x_sb[:, eo, ch * CH : (ch + 1) * CH],
                            start=(eo == 0),
                            stop=(eo == EO - 1),
                        )
                    nc.scalar.activation(
                        dst[:, do, ch * CH : (ch + 1) * CH],
                        ps,
                        mybir.ActivationFunctionType.Identity,
                        bias=bias_t[:, do : do + 1],
                        scale=1.0,
                    )

        # rest of kT, then qT (scores need them first).  wk1 rides the
        # scalar+sync rings (both drain before evictions start); later chunks
        # use sync+gpsimd to keep the Scalar queue free for activations.
        wt = load_w_chunk(wk_r, 1, "wk1", rings=(nc.scalar, nc.sync))
        project_chunk(wt, 1, kT_sb, bk_t)
        for c in range(NCH):
            wt = load_w_chunk(wq_r, c, f"wq{c}")
            project_chunk(wt, c, qT_sb, bq_t)

        # ---- all scores + exp (Act drains one tile behind the PE) ----
        rinvs = []
        for i in range(SO):
            nk = i + 1
            kw = nk * P
            ps_s = sc_ps.tile([P, S], F32, tag="scores", name="ps_s")
            nfull = kw // CH
            rem = kw - nfull * CH
            for ch in range(nfull + (1 if rem else 0)):
                w = CH if ch < nfull else rem
                for do in range(DO):
                    nc.tensor.matmul(
                        ps_s[:, ch * CH : ch * CH + w],
                        lhsT=qT_sb[:, do, i * P : (i + 1) * P],
                        rhs=kT_sb[:, do, ch * CH : ch * CH + w],
                        start=(do == 0),
                        stop=(do == DO - 1),
                    )
            # additive causal mask on the diagonal block
            nc.vector.tensor_tensor(
                ps_s[:, i * P : (i + 1) * P],
                ps_s[:, i * P : (i + 1) * P],
                cmask,
                mybir.AluOpType.add,
            )
            # p = exp(scores); no max subtraction; fused row-sum
            lsum = small.tile([P, 1], F32, tag="lsum", name=f"lsum{i}")
            nc.scalar.activation(
                p_all[:, i, 0:kw],
                ps_s[:, :kw],
                mybir.ActivationFunctionType.Exp,
                bias=0.0,
                scale=1.0,
                accum_out=lsum,
            )
            rinv = small.tile([P, 1], F32, tag="rinv", name=f"rinv{i}")
            nc.vector.reciprocal(rinv, lsum)
            rinvs.append(rinv)

        # ---- V[s_part, so, d] = x.T @ WvT (+bv along free dim) ----
        for c in range(NCH):
            wt = load_w_chunk(wv_r, c, f"wv{c}")
            for so in range(SO):
                ps = acc_ps.tile([P, CH], F32, tag="acc", name="ps")
                for eo in range(EO):
                    nc.tensor.matmul(
                        ps,
                        lhsT=x_sb[:, eo, so * P : (so + 1) * P],
                        rhs=wt[:, eo, :],
                        start=(eo == 0),
                        stop=(eo == EO - 1),
                    )
                nc.vector.tensor_tensor(
                    v_sb[:, so, c * CH : (c + 1) * CH],
                    ps,
                    bv_b[:, c * CH : (c + 1) * CH],
                    mybir.AluOpType.add,
                )

        # ---- transpose + AV per q-tile (all deps already resolved) ----
        # Descending i: the big early tiles hide the transpose->copy->AV
        # pipeline-fill latency, and the kernel ends on the smallest tile so
        # the final evict+DMA chain is short.
        for i in reversed(range(SO)):
            nk = i + 1
            rinv = rinvs[i]
            pT = ppool.tile([P, S], BF16, tag="pT", name="pT")
            for j in range(nk):
                ps_t = tr_ps.tile([P, P], BF16, tag="tr", name="ps_t")
                nc.tensor.transpose(ps_t, p_all[:, i, j * P : (j + 1) * P], identity)
                nc.vector.tensor_copy(pT[:, j * P : (j + 1) * P], ps_t)

            out_sb = ppool.tile([P, D], BF16, tag="out", name="out_sb")
            last = i == 0
            for c2 in range(NCH):
                ps_o = acc_ps.tile([P, CH], F32, tag="acc", name="ps_o")
                for j in range(nk):
                    nc.tensor.matmul(
                        ps_o,
                        lhsT=pT[:, j * P : (j + 1) * P],
                        rhs=v_sb[:, j, c2 * CH : (c2 + 1) * CH],
                        start=(j == 0),
                        stop=(j == nk - 1),
                    )
                base = c2 * CH
                if last and c2 == NCH - 1:
                    # final tile: split eviction across DVE+Act, one DMA
                    # issue per output ring, to shorten the tail drain
                    half = CH // 2
                    nc.vector.tensor_scalar_mul(
                        out_sb[:, base : base + half], ps_o[:, 0:half], rinv
                    )
                    nc.scalar.activation(
                        out_sb[:, base + half : base + CH],
                        ps_o[:, half:CH],
                        mybir.ActivationFunctionType.Identity,
                        bias=0.0,
                        scale=rinv,
                    )
                    nc.gpsimd.dma_start(
                        out[i * P : (i + 1) * P, base : base + half],
                        out_sb[:, base : base + half],
                    )
                    nc.sync.dma_start(
                        out[i * P : (i + 1) * P, base + half : base + CH],
                        out_sb[:, base + half : base + CH],
                    )
                else:
                    # alternate evictions DVE/Act and output rings
                    if c2 % 2 == 0:
                        nc.vector.tensor_scalar_mul(
                            out_sb[:, base : base + CH], ps_o, rinv
                        )
                    else:
                        nc.scalar.activation(
                            out_sb[:, base : base + CH],
                            ps_o,
                            mybir.ActivationFunctionType.Identity,
                            bias=0.0,
                            scale=rinv,
                        )
                    ring = nc.gpsimd if (i * NCH + c2) % 2 == 0 else nc.sync
                    ring.dma_start(
                        out[i * P : (i + 1) * P, base : base + CH],
                        out_sb[:, base : base + CH],
                    )

    nc.compile()
    return nc


_NC_CACHE = None


def get_program():
    global _NC_CACHE
    if _NC_CACHE is None:
        _NC_CACHE = build_program()
    return _NC_CACHE


def make_in_maps(x, Wq, bq, Wk, bk, Wv, bv):
    x = np.asarray(x, dtype=np.float32)
    wqT = np.ascontiguousarray(
        (np.asarray(Wq, dtype=np.float32).T * np.float32(SCALE)).astype(
            ml_dtypes.bfloat16
        )
    )
    wkT = np.ascontiguousarray(np.asarray(Wk, dtype=np.float32).T.astype(ml_dtypes.bfloat16))
    wvT = np.ascontiguousarray(np.asarray(Wv, dtype=np.float32).T.astype(ml_dtypes.bfloat16))
    bqs = np.asarray(bq, dtype=np.float32) * np.float32(SCALE)
    bk = np.asarray(bk, dtype=np.float32)
    bv = np.asarray(bv, dtype=np.float32)
    in_maps = []
    for b in range(B):
        in_maps.append(
            {
                "xT": np.ascontiguousarray(x[b].T.astype(ml_dtypes.bfloat16)),
                "wqT": wqT,
                "wkT": wkT,
                "wvT": wvT,
                "bqs": bqs,
                "bk": bk,
                "bv": bv,
            }
        )
    return in_maps


def run_on_hw(in_maps, trace=False, **kwargs):
    nc = get_program()
    return run_bass_kernel_spmd(
        nc, in_maps, core_ids=list(range(B)), trace=trace, **kwargs
    )


def kernel(x, Wq, bq, Wk, bk, Wv, bv):
    in_maps = make_in_maps(x, Wq, bq, Wk, bk, Wv, bv)
    res = run_on_hw(in_maps)
    return np.stack(
        [res.results[b]["out"].astype(np.float32) for b in range(B)], axis=0
    )
